# revision 4
# baseline (speedup 1.0000x reference)
"""DeMBR multi-behavior LightGCN kernel for Trainium2 (8 NeuronCores).

v3 strategy (per dense behavior, each [N,N] relation matrix R):
  - Hybrid block sharding (as v2). Core k holds TWO fp8(e3m4) views of R:
      At = R[rows 512k:512k+512, :].T   as [128, 32ic, 512u]  (item-partition)
      Ac = R[:, cols 512k:512k+512]     as [128, 32uc, 512j]  (user-partition)
  - COLUMN-TILED matmul pairs: the stationary embeddings are only 64 wide,
    so the u-side (i0.T @ At) runs in PE array columns 0-63 and the i-side
    (u0.T @ Ac) runs concurrently in columns 64-127 (tile_position inferred
    from the psum output base partition).  One [128,512] psum bank holds
    both halves.  This ~2x's effective PE throughput vs v2.
  - Layer-1 psum [128,512] (u1_un.T | i1_un.T stacked) is copied to SBUF,
    PE-transposed as four full 128x128 blocks (u and i halves transposed
    together), deg-scaled into bf16 zi [128, 4, 128] = [u1 | i1] natural.
    zi is BOTH the u1/i1 output (host reads it directly) and the AllGather
    payload - one buffer, no separate f32 layer-1 export.
  - Per-behavior AllGather (bf16, 128 KB in / 1 MB out).  The cc stream has
    a large one-time setup/skew barrier which a warmup AllGather (fired in
    the first microseconds) absorbs; the per-behavior AGs then pipeline
    behind it while layer-1 compute streams.
  - P2: stationaries come from the gathered G tile (natural layout bf16);
    col-tiled pairs again; raw psum [128,512] exported f32; host applies
    deg scaling and the /2 averaging.
  - Loads: one 2 MB dma_start per At/Ac tensor (b=0 split in halves so the
    first matmuls start early), At on the sync HWDGE queue, Ac on the
    scalar queue.  Gather readbacks ride sync after the loads drain;
    P2 exports ride scalar; zi export + AG input writes go via gpsimd
    (SWDGE) so they never queue behind the bulk loads.
  - deg_u/deg_i are computed on the host in f64 from the original f32 R
    (exactly matching the reference); all-ones virtual matrices are
    detected and computed analytically.

kernel(**inputs) takes the full unsharded inputs and returns [14, 4096, 64].
"""

import os
import numpy as np
import ml_dtypes

EPS = 1e-8
N, D = 4096, 64
P = 128
NCORES = 8
BLK = N // NCORES           # 512 users/items per core block
NB_CH = BLK // P            # 4 chunks per block
NCH = N // P                # 32 chunks over the full dim

_BF16 = ml_dtypes.bfloat16
_E3M4 = ml_dtypes.float8_e3m4


# --------------------------------------------------------------------------
# device program
# --------------------------------------------------------------------------

def build_program(nb):
    """Build + bacc-compile the SPMD program for `nb` dense behaviors."""
    import concourse.bass as bass  # noqa: F401  (registers types)
    import concourse.mybir as mybir
    import concourse.tile as tile
    from concourse import bacc
    from concourse.masks import make_identity

    f32, bf16 = mybir.dt.float32, mybir.dt.bfloat16
    fp8 = mybir.dt.float8e3
    ALU = mybir.AluOpType

    nc = bacc.Bacc("TRN2", target_bir_lowering=False, debug=False,
                   num_devices=NCORES)

    At_in = [nc.dram_tensor(f"At{b}", [P, NCH, BLK], fp8, kind="ExternalInput")
             for b in range(nb)]
    Ac_in = [nc.dram_tensor(f"Ac{b}", [P, NCH, BLK], fp8, kind="ExternalInput")
             for b in range(nb)]
    i0s_in = nc.dram_tensor("i0s", [P, NCH, D], bf16, kind="ExternalInput")
    u0s_in = nc.dram_tensor("u0s", [P, NCH, D], bf16, kind="ExternalInput")
    # per-behavior reciprocal degree slices for this core's blocks
    rud_in = nc.dram_tensor("rud", [P, nb, NB_CH], f32, kind="ExternalInput")
    rid_in = nc.dram_tensor("rid", [P, nb, NB_CH], f32, kind="ExternalInput")

    # zi[p, f, 0:64] = u1[user f*128+p of block], zi[p, f, 64:128] = i1
    zi_out = [nc.dram_tensor(f"zi{b}", [P, NB_CH, 2 * D], bf16,
                             kind="ExternalOutput") for b in range(nb)]
    # p2[0:64, :] = u2_un.T, p2[64:128, :] = i2_un.T (raw, host scales)
    p2_out = [nc.dram_tensor(f"p2_{b}", [P, BLK], f32,
                             kind="ExternalOutput") for b in range(nb)]
    warm_out = nc.dram_tensor("warm", [P, 2], f32, kind="ExternalOutput")

    rg = [list(range(NCORES))]

    with tile.TileContext(nc) as tc:
        with (
            tc.tile_pool(name="big", bufs=nb) as pbig,
            tc.tile_pool(name="gat", bufs=nb) as pgat,
            tc.tile_pool(name="sm", bufs=2) as psm,
            tc.tile_pool(name="one", bufs=1) as pone,
            tc.tile_pool(name="mm", bufs=2, space="PSUM") as pmm,
            tc.tile_pool(name="tr", bufs=2, space="PSUM") as ptr,
            tc.tile_pool(name="wp", bufs=1, space="PSUM") as pwp,
            tc.tile_pool(name="dr2", bufs=4, space="DRAM") as pdr,
        ):
            # warmup collective fired immediately (input is an uninitialized
            # DRAM tile - AllGather/bypass only moves bytes): absorbs the
            # one-time cc-stream setup + cross-core skew barrier under the
            # load/P1 window
            w_in = pdr.tile([P, 1], f32, tag="w_in", name="w_in")
            w_out = pdr.tile([NCORES, P, 1], f32, tag="w_out", name="w_out",
                             addr_space="Shared")
            nc.gpsimd.collective_compute(
                "AllGather", ALU.bypass, replica_groups=rg,
                ins=[w_in.opt()], outs=[w_out.opt()])

            ident = pone.tile([P, P], f32)
            make_identity(nc, ident[:])

            # ~4us of dummy matmuls while the first loads stream: tips the
            # PE's HAM activity window into the 2.4 GHz state before the real
            # work arrives (results exported so DCE keeps them).  Emitted at
            # high priority so the Tile scheduler keeps them at the front of
            # the PE queue.
            with tc.high_priority():
                Pw = pwp.tile([P, P], f32, tag="WARM", name="Pw", bufs=1)
                for it in range(24):
                    nc.tensor.matmul(Pw[:], ident[:], ident[:],
                                     start=(it == 0), stop=(it == 23))
                warm_sb = pone.tile([P, 2], f32)
                nc.vector.tensor_copy(out=warm_sb[:], in_=Pw[:, 0:2])
                nc.gpsimd.dma_start(out=warm_out.ap(), in_=warm_sb[:])

            i0s = pone.tile([P, NCH, D], bf16)
            nc.sync.dma_start(out=i0s[:], in_=i0s_in[:])
            u0s = pone.tile([P, NCH, D], bf16)
            nc.scalar.dma_start(out=u0s[:], in_=u0s_in[:])
            rud = pone.tile([P, nb, NB_CH], f32)
            nc.sync.dma_start(out=rud[:], in_=rud_in[:])
            rid = pone.tile([P, nb, NB_CH], f32)
            nc.scalar.dma_start(out=rid[:], in_=rid_in[:])

            at_tiles, ac_tiles, state = {}, {}, {}

            def load(b):
                # At on the sync HWDGE ring, Ac on the scalar (ACT) ring:
                # the two physical HWDGE queues stream in parallel.  Big
                # transfers amortize the per-DMA fixed cost; behavior 0 is
                # split in halves so its first chunks land early.
                At = pbig.tile([P, NCH, BLK], fp8, tag="At", name=f"At{b}")
                Ac = pbig.tile([P, NCH, BLK], fp8, tag="Ac", name=f"Ac{b}")
                nparts = 2 if b == 0 else 1
                q = NCH // nparts
                for g in range(nparts):
                    sl = slice(g * q, (g + 1) * q)
                    nc.sync.dma_start(out=At[:, sl, :], in_=At_in[b][:, sl, :])
                    nc.scalar.dma_start(out=Ac[:, sl, :],
                                        in_=Ac_in[b][:, sl, :])
                at_tiles[b], ac_tiles[b] = At, Ac

            def phase1(b):
                At, Ac = at_tiles[b], ac_tiles[b]
                # col-tiled pairs: u-side -> psum[0:64], i-side -> [64:128]
                Pp = pmm.tile([P, BLK], f32, tag="PC", name=f"P1_{b}")
                for c in range(NCH):
                    nc.tensor.matmul(Pp[0:D, :], i0s[:, c, :], At[:, c, :],
                                     start=(c == 0), stop=(c == NCH - 1))
                    nc.tensor.matmul(Pp[D:2 * D, :], u0s[:, c, :],
                                     Ac[:, c, :],
                                     start=(c == 0), stop=(c == NCH - 1))
                S = psm.tile([P, BLK], f32, tag="S", name=f"S{b}")
                nc.vector.tensor_copy(out=S[:], in_=Pp[:])

                # transpose the [128,128] blocks whole: cols 0:64 become u1
                # natural, cols 64:128 become i1 natural
                PT = ptr.tile([P, NB_CH, P], f32, tag="PT", name=f"PT{b}")
                for f in range(NB_CH):
                    nc.tensor.transpose(PT[:, f, :],
                                        S[:, f * P:(f + 1) * P],
                                        ident[:])
                zi = psm.tile([P, NB_CH, 2 * D], bf16, tag="zi", name=f"zi{b}")
                for f in range(NB_CH):
                    nc.vector.tensor_scalar_mul(out=zi[:, f, 0:D],
                                                in0=PT[:, f, 0:D],
                                                scalar1=rud[:, b, f:f + 1])
                    nc.vector.tensor_scalar_mul(out=zi[:, f, D:2 * D],
                                                in0=PT[:, f, D:2 * D],
                                                scalar1=rid[:, b, f:f + 1])
                # zi is both the layer-1 output and the AllGather payload
                nc.gpsimd.dma_start(out=zi_out[b].ap(), in_=zi[:])
                z_in = pdr.tile([P, NB_CH, 2 * D], bf16, tag="z_in",
                                name=f"z_in{b}")
                nc.gpsimd.dma_start(out=z_in[:], in_=zi[:])
                state[b] = z_in

            def gather(b):
                z_in = state.pop(b)
                z_out = pdr.tile([NCORES, P, NB_CH, 2 * D], bf16,
                                 tag="z_out", name=f"z_out{b}",
                                 addr_space="Shared")
                nc.gpsimd.collective_compute(
                    "AllGather", ALU.bypass, replica_groups=rg,
                    ins=[z_in.opt()], outs=[z_out.opt()])
                G = pgat.tile([P, NCORES, NB_CH, 2 * D], bf16, tag="G",
                              name=f"G{b}")
                nc.sync.dma_start(
                    out=G[:],
                    in_=z_out[:].rearrange("c p f x -> p c f x"))
                state[b] = G

            def phase2(b):
                At, Ac = at_tiles.pop(b), ac_tiles.pop(b)
                G = state.pop(b)
                Pp = pmm.tile([P, BLK], f32, tag="PC", name=f"P2_{b}")
                for c in range(NCH):
                    nc.tensor.matmul(Pp[0:D, :],
                                     G[:, c // NB_CH, c % NB_CH, D:2 * D],
                                     At[:, c, :],
                                     start=(c == 0), stop=(c == NCH - 1))
                    nc.tensor.matmul(Pp[D:2 * D, :],
                                     G[:, c // NB_CH, c % NB_CH, 0:D],
                                     Ac[:, c, :],
                                     start=(c == 0), stop=(c == NCH - 1))
                T = psm.tile([P, BLK], f32, tag="T", name=f"T{b}")
                nc.vector.tensor_copy(out=T[:], in_=Pp[:])
                nc.scalar.dma_start(out=p2_out[b].ap(), in_=T[:])

            for b in range(nb):
                load(b)
            for b in range(nb):
                phase1(b)
                gather(b)
            for b in range(nb):
                phase2(b)

    nc.compile()
    return nc


# --------------------------------------------------------------------------
# host-side helpers
# --------------------------------------------------------------------------

def _chunk_part(x):
    """[4096, C] -> [128, 32, C] with row = c*128 + p."""
    return np.ascontiguousarray(
        x.reshape(NCH, P, x.shape[1]).transpose(1, 0, 2))


def host_prep_behavior(R):
    """Quantize to e3m4 + exact f64 degree reciprocals."""
    Rq = R.astype(_E3M4)
    deg_u = R.sum(axis=1, dtype=np.float64)
    deg_i = R.sum(axis=0, dtype=np.float64)
    ru = (1.0 / (deg_u + EPS)).astype(np.float32)
    ri = (1.0 / (deg_i + EPS)).astype(np.float32)
    return Rq, ru, ri, deg_u, deg_i


def _core_layouts(Rq, k):
    """Per-core At/Ac tiles in [128, 32, 512] linear-DMA order."""
    rows = Rq[k * BLK:(k + 1) * BLK, :]          # [512, 4096]
    # At[p, ic, u] = rows[u, ic*128+p]
    At = np.ascontiguousarray(
        rows.T.reshape(NCH, P, BLK).transpose(1, 0, 2))
    cols = Rq[:, k * BLK:(k + 1) * BLK]          # [4096, 512]
    # Ac[p, uc, j] = cols[uc*128+p, j]
    Ac = np.ascontiguousarray(
        cols.reshape(NCH, P, BLK).transpose(1, 0, 2))
    return At, Ac


def prep_in_maps(prepped, u0, i0):
    i0s = _chunk_part(i0.astype(_BF16))
    u0s = _chunk_part(u0.astype(_BF16))
    in_maps = []
    for k in range(NCORES):
        sl = slice(k * BLK, (k + 1) * BLK)
        m = {"i0s": i0s, "u0s": u0s}
        # rud[p, b, f] = 1/deg_u[k*512 + f*128 + p] for behavior b
        m["rud"] = np.ascontiguousarray(np.stack(
            [p[1][sl].reshape(NB_CH, P).T for p in prepped], axis=1))
        m["rid"] = np.ascontiguousarray(np.stack(
            [p[2][sl].reshape(NB_CH, P).T for p in prepped], axis=1))
        for b, p in enumerate(prepped):
            At, Ac = _core_layouts(p[0], k)
            m[f"At{b}"] = At
            m[f"Ac{b}"] = Ac
        in_maps.append(m)
    return in_maps


def assemble_dense(results, prepped, nb):
    """Per-behavior (u_acc [N,D], i_acc [N,D]) from per-core outputs."""
    out = []
    for b in range(nb):
        _, ru, ri, _, _ = prepped[b]
        u_acc = np.empty((N, D), np.float32)
        i_acc = np.empty((N, D), np.float32)
        for k in range(NCORES):
            sl = slice(k * BLK, (k + 1) * BLK)
            zi = results[k][f"zi{b}"].astype(np.float32)
            u1 = zi[:, :, 0:D].transpose(1, 0, 2).reshape(BLK, D)
            i1 = zi[:, :, D:2 * D].transpose(1, 0, 2).reshape(BLK, D)
            p2 = results[k][f"p2_{b}"]
            u2 = p2[0:D, :].T * ru[sl][:, None]
            i2 = p2[D:2 * D, :].T * ri[sl][:, None]
            u_acc[sl] = (u1 + u2) * np.float32(0.5)
            i_acc[sl] = (i1 + i2) * np.float32(0.5)
        out.append((u_acc, i_acc))
    return out


def ones_behavior(u0, i0):
    """Analytic LightGCN-2-layer outputs when R is all-ones [N, N]."""
    s_i = i0.astype(np.float64).sum(axis=0)
    s_u = u0.astype(np.float64).sum(axis=0)
    d = N + EPS
    u_row = (s_i / d + s_u * N / (d * d)) * 0.5
    i_row = (s_u / d + s_i * N / (d * d)) * 0.5
    u = np.broadcast_to(u_row.astype(np.float32), (N, D)).copy()
    it = np.broadcast_to(i_row.astype(np.float32), (N, D)).copy()
    return u, it


# --------------------------------------------------------------------------
# cached device runner (compile once per behavior-count, run many)
# --------------------------------------------------------------------------

_RUNNERS = {}


class _Runner:
    def __init__(self, nb):
        self.nb = nb
        self.nc = build_program(nb)
        self._jitted = None
        self._meta = None

    def _prep_jit(self):
        import jax
        import numpy as _np
        from jax.sharding import Mesh, PartitionSpec
        from jax.experimental.shard_map import shard_map
        from concourse import bass2jax
        from concourse.bass2jax import _bass_exec_p, partition_id_tensor
        import concourse.mybir as mybir

        bass2jax.install_neuronx_cc_hook()
        nc = self.nc
        partition_name = (nc.partition_id_tensor.name
                          if nc.partition_id_tensor else None)
        in_names, out_names, out_avals, zero_shapes = [], [], [], []
        for alloc in nc.m.functions[0].allocations:
            if not isinstance(alloc, mybir.MemoryLocationSet):
                continue
            name = alloc.memorylocations[0].name
            if alloc.kind == "ExternalInput":
                if name != partition_name:
                    in_names.append(name)
            elif alloc.kind == "ExternalOutput":
                shape = tuple(alloc.tensor_shape)
                dtype = mybir.dt.np(alloc.dtype)
                out_names.append(name)
                out_avals.append(jax.core.ShapedArray(shape, dtype))
                zero_shapes.append((shape, dtype))
        n_params = len(in_names)
        full_in_names = list(in_names) + list(out_names)
        if partition_name is not None:
            full_in_names.append(partition_name)

        def _body(*args):
            operands = list(args)
            if partition_name is not None:
                operands.append(partition_id_tensor())
            outs = _bass_exec_p.bind(
                *operands,
                out_avals=tuple(out_avals),
                in_names=tuple(full_in_names),
                out_names=tuple(out_names),
                lowering_input_output_aliases=(),
                sim_require_finite=True,
                sim_require_nnan=True,
                nc=nc,
            )
            return tuple(outs)

        devices = jax.devices()[:NCORES]
        mesh = Mesh(_np.asarray(devices), ("core",))
        n_outs = len(out_names)
        in_specs = (PartitionSpec("core"),) * (n_params + n_outs)
        out_specs = (PartitionSpec("core"),) * n_outs
        donate = tuple(range(n_params, n_params + n_outs))
        self._jitted = jax.jit(
            shard_map(_body, mesh=mesh, in_specs=in_specs,
                      out_specs=out_specs, check_rep=False),
            donate_argnums=donate, keep_unused=True)
        self._meta = (in_names, out_names, out_avals, zero_shapes, n_params)

    def run(self, in_maps):
        if self._jitted is None:
            self._prep_jit()
        import numpy as _np
        in_names, out_names, out_avals, zero_shapes, n_params = self._meta
        concat_in = [
            _np.concatenate([_np.asarray(in_maps[c][nm]) for c in range(NCORES)],
                            axis=0)
            for nm in in_names]
        concat_zeros = [_np.zeros((NCORES * s[0], *s[1:]), dt)
                        for (s, dt) in zero_shapes]
        out_arrs = self._jitted(*concat_in, *concat_zeros)
        results = []
        for c in range(NCORES):
            results.append({
                nm: _np.asarray(out_arrs[i]).reshape(
                    NCORES, *out_avals[i].shape)[c]
                for i, nm in enumerate(out_names)})
        return results

    def run_traced(self, in_maps, tmpdir=None):
        """Run through run_bass_kernel_spmd with NTFF tracing (recompiles)."""
        _install_trace_shims()
        from concourse.bass_utils import run_bass_kernel_spmd
        return run_bass_kernel_spmd(self.nc, in_maps,
                                    core_ids=list(range(NCORES)),
                                    trace=True, tmpdir=tmpdir)


def _install_trace_shims():
    """This image's antenv lacks axon_hooks (the NTFF-hook registry) and has
    no artifact bucket; recreate the hook from the boot recipe and make
    artifact upload a local no-op."""
    import sys, types, importlib.util

    if "antenv.axon_hooks" not in sys.modules:
        mod = types.ModuleType("antenv.axon_hooks")
        mod._hook = None

        def set_axon_ntff_profile_hook(h):
            mod._hook = h

        def get_axon_ntff_profile_hook():
            return mod._hook

        mod.set_axon_ntff_profile_hook = set_axon_ntff_profile_hook
        mod.get_axon_ntff_profile_hook = get_axon_ntff_profile_hook
        import antenv
        sys.modules["antenv.axon_hooks"] = mod
        antenv.axon_hooks = mod

        spec = importlib.util.spec_from_file_location(
            "trn_boot_shim", "/root/.axon_site/trn_agent_boot/trn_boot.py")
        boot = importlib.util.module_from_spec(spec)
        spec.loader.exec_module(boot)
        hook = boot._ntff_profile_via_ctypes("/opt/axon/libaxon_pjrt.so")
        mod._hook = hook

    import concourse.bass_utils as bu
    if not getattr(bu.upload_artifacts, "_is_local_shim", False):
        def _local_upload(tmpdir):
            return tmpdir
        _local_upload._is_local_shim = True
        bu.upload_artifacts = _local_upload


def get_runner(nb):
    if nb not in _RUNNERS:
        _RUNNERS[nb] = _Runner(nb)
    return _RUNNERS[nb]


# --------------------------------------------------------------------------
# entry point
# --------------------------------------------------------------------------

def _is_ones(a):
    return a[0, 0] == 1.0 and bool(np.all(a == np.float32(1.0)))


def kernel(**inputs):
    inputs = {k: np.asarray(v) for k, v in inputs.items()}
    u0 = np.ascontiguousarray(inputs["user_embedding"], dtype=np.float32)
    i0 = np.ascontiguousarray(inputs["item_embedding"], dtype=np.float32)

    real_names = ["R_click", "R_fav", "R_cart", "R_buy"]
    virt_names = [("M_click", "add_click"), ("M_fav", "add_fav"),
                  ("M_cart", "add_cart")]
    mats = [np.asarray(inputs[n], dtype=np.float32) for n in real_names]
    mats += [np.asarray(inputs[m], dtype=np.float32) for m, _ in virt_names]

    dense_idx = [j for j, a in enumerate(mats) if not _is_ones(a)]
    per_behavior = [None] * 7

    if dense_idx:
        nb = len(dense_idx)
        runner = get_runner(nb)
        prepped = [host_prep_behavior(mats[j]) for j in dense_idx]
        in_maps = prep_in_maps(prepped, u0, i0)
        results = runner.run(in_maps)
        dense = assemble_dense(results, prepped, nb)
        for pos, j in enumerate(dense_idx):
            per_behavior[j] = dense[pos]

    ones_cache = None
    for j, a in enumerate(mats):
        if per_behavior[j] is None:
            if ones_cache is None:
                ones_cache = ones_behavior(u0, i0)
            per_behavior[j] = ones_cache
    ur = [per_behavior[j][0] for j in range(4)]
    ir = [per_behavior[j][1] for j in range(4)]
    uv = [per_behavior[4 + j][0] + np.asarray(inputs[virt_names[j][1]],
                                              dtype=np.float32)
          for j in range(3)]
    iv = [per_behavior[4 + j][1] for j in range(3)]

    out = np.concatenate(
        [np.stack(ur), np.stack(ir), np.stack(uv), np.stack(iv)], axis=0)
    return np.ascontiguousarray(out, dtype=np.float32)


# revision 7
# speedup vs baseline: 8.0332x; 8.0332x over previous
"""DeMBR multi-behavior LightGCN kernel for Trainium2 (8 NeuronCores).

v3 strategy (per dense behavior, each [N,N] relation matrix R):
  - Hybrid block sharding (as v2). Core k holds TWO fp8(e3m4) views of R:
      At = R[rows 512k:512k+512, :].T   as [128, 32ic, 512u]  (item-partition)
      Ac = R[:, cols 512k:512k+512]     as [128, 32uc, 512j]  (user-partition)
  - COLUMN-TILED matmul pairs: the stationary embeddings are only 64 wide,
    so the u-side (i0.T @ At) runs in PE array columns 0-63 and the i-side
    (u0.T @ Ac) runs concurrently in columns 64-127 (tile_position inferred
    from the psum output base partition).  One [128,512] psum bank holds
    both halves.  This ~2x's effective PE throughput vs v2.
  - Layer-1 psum [128,512] (u1_un.T | i1_un.T stacked) is copied to SBUF,
    PE-transposed as four full 128x128 blocks (u and i halves transposed
    together), deg-scaled into bf16 zi [128, 4, 128] = [u1 | i1] natural.
    zi is BOTH the u1/i1 output (host reads it directly) and the AllGather
    payload - one buffer, no separate f32 layer-1 export.
  - Per-behavior AllGather (bf16, 128 KB in / 1 MB out).  The cc stream has
    a large one-time setup/skew barrier which a warmup AllGather (fired in
    the first microseconds) absorbs; the per-behavior AGs then pipeline
    behind it while layer-1 compute streams.
  - P2: stationaries come from the gathered G tile (natural layout bf16);
    col-tiled pairs again; raw psum [128,512] exported f32; host applies
    deg scaling and the /2 averaging.
  - Loads: one 2 MB dma_start per At/Ac tensor (b=0 split in halves so the
    first matmuls start early), At on the sync HWDGE queue, Ac on the
    scalar queue.  Gather readbacks ride sync after the loads drain;
    P2 exports ride scalar; zi export + AG input writes go via gpsimd
    (SWDGE) so they never queue behind the bulk loads.
  - deg_u/deg_i are computed on the host in f64 from the original f32 R
    (exactly matching the reference); all-ones virtual matrices are
    detected and computed analytically.

kernel(**inputs) takes the full unsharded inputs and returns [14, 4096, 64].
"""

import os
import numpy as np
import ml_dtypes

EPS = 1e-8
N, D = 4096, 64
P = 128
NCORES = 8
BLK = N // NCORES           # 512 users/items per core block
NB_CH = BLK // P            # 4 chunks per block
NCH = N // P                # 32 chunks over the full dim

_BF16 = ml_dtypes.bfloat16
_E3M4 = ml_dtypes.float8_e3m4


# --------------------------------------------------------------------------
# device program
# --------------------------------------------------------------------------

def build_program(nb):
    """Build + bacc-compile the SPMD program for `nb` dense behaviors."""
    import concourse.bass as bass  # noqa: F401  (registers types)
    import concourse.mybir as mybir
    import concourse.tile as tile
    from concourse import bacc
    from concourse.masks import make_identity

    f32, bf16 = mybir.dt.float32, mybir.dt.bfloat16
    fp8 = mybir.dt.float8e3
    ALU = mybir.AluOpType

    nc = bacc.Bacc("TRN2", target_bir_lowering=False, debug=False,
                   num_devices=NCORES)

    At_in = [nc.dram_tensor(f"At{b}", [P, NCH, BLK], fp8, kind="ExternalInput")
             for b in range(nb)]
    Ac_in = [nc.dram_tensor(f"Ac{b}", [P, NCH, BLK], fp8, kind="ExternalInput")
             for b in range(nb)]
    i0s_in = nc.dram_tensor("i0s", [P, NCH, D], bf16, kind="ExternalInput")
    u0s_in = nc.dram_tensor("u0s", [P, NCH, D], bf16, kind="ExternalInput")
    # per-behavior reciprocal degree slices for this core's blocks
    rud_in = nc.dram_tensor("rud", [P, nb, NB_CH], f32, kind="ExternalInput")
    rid_in = nc.dram_tensor("rid", [P, nb, NB_CH], f32, kind="ExternalInput")

    # zi[p, f, 0:64] = u1[user f*128+p of block], zi[p, f, 64:128] = i1
    zi_out = [nc.dram_tensor(f"zi{b}", [P, NB_CH, 2 * D], bf16,
                             kind="ExternalOutput") for b in range(nb)]
    # p2[0:64, :] = u2_un.T, p2[64:128, :] = i2_un.T (raw, host scales)
    p2_out = [nc.dram_tensor(f"p2_{b}", [P, BLK], f32,
                             kind="ExternalOutput") for b in range(nb)]
    warm_out = nc.dram_tensor("warm", [P, 2], f32, kind="ExternalOutput")

    rg = [list(range(NCORES))]

    with tile.TileContext(nc) as tc:
        with (
            tc.tile_pool(name="big", bufs=nb) as pbig,
            tc.tile_pool(name="gat", bufs=nb) as pgat,
            tc.tile_pool(name="sm", bufs=2) as psm,
            tc.tile_pool(name="one", bufs=1) as pone,
            tc.tile_pool(name="mm", bufs=2, space="PSUM") as pmm,
            tc.tile_pool(name="tr", bufs=2, space="PSUM") as ptr,
            tc.tile_pool(name="wp", bufs=1, space="PSUM") as pwp,
            tc.tile_pool(name="dr2", bufs=4, space="DRAM") as pdr,
        ):
            # warmup collective fired immediately (input is an uninitialized
            # DRAM tile - AllGather/bypass only moves bytes): absorbs the
            # one-time cc-stream setup + cross-core skew barrier under the
            # load/P1 window
            w_in = pdr.tile([P, 1], f32, tag="w_in", name="w_in")
            w_out = pdr.tile([NCORES, P, 1], f32, tag="w_out", name="w_out",
                             addr_space="Shared")
            nc.gpsimd.collective_compute(
                "AllGather", ALU.bypass, replica_groups=rg,
                ins=[w_in.opt()], outs=[w_out.opt()])

            ident = pone.tile([P, P], f32)
            make_identity(nc, ident[:])

            # ~4us of dummy matmuls while the first loads stream: tips the
            # PE's HAM activity window into the 2.4 GHz state before the real
            # work arrives (results exported so DCE keeps them; the export
            # itself is emitted at the very end so it never head-blocks a
            # queue).
            Pw = pwp.tile([P, P], f32, tag="WARM", name="Pw", bufs=1)
            for it in range(24):
                nc.tensor.matmul(Pw[:], ident[:], ident[:],
                                 start=(it == 0), stop=(it == 23))
            warm_sb = pone.tile([P, 2], f32)
            nc.vector.tensor_copy(out=warm_sb[:], in_=Pw[:, 0:2])

            i0s = pone.tile([P, NCH, D], bf16)
            nc.sync.dma_start(out=i0s[:], in_=i0s_in[:])
            u0s = pone.tile([P, NCH, D], bf16)
            nc.scalar.dma_start(out=u0s[:], in_=u0s_in[:])
            rud = pone.tile([P, nb, NB_CH], f32)
            nc.sync.dma_start(out=rud[:], in_=rud_in[:])
            rid = pone.tile([P, nb, NB_CH], f32)
            nc.scalar.dma_start(out=rid[:], in_=rid_in[:])

            at_tiles, ac_tiles, state = {}, {}, {}

            def load(b):
                # At on the sync HWDGE ring, Ac on the scalar (ACT) ring:
                # the two physical HWDGE queues stream in parallel.  Big
                # transfers amortize the per-DMA fixed cost; behavior 0 is
                # split in halves so its first chunks land early.
                At = pbig.tile([P, NCH, BLK], fp8, tag="At", name=f"At{b}")
                Ac = pbig.tile([P, NCH, BLK], fp8, tag="Ac", name=f"Ac{b}")
                nparts = 2 if b == 0 else 1
                q = NCH // nparts
                for g in range(nparts):
                    sl = slice(g * q, (g + 1) * q)
                    nc.sync.dma_start(out=At[:, sl, :], in_=At_in[b][:, sl, :])
                    nc.scalar.dma_start(out=Ac[:, sl, :],
                                        in_=Ac_in[b][:, sl, :])
                at_tiles[b], ac_tiles[b] = At, Ac

            def phase1(b):
                At, Ac = at_tiles[b], ac_tiles[b]
                # col-tiled pairs: u-side -> psum[0:64], i-side -> [64:128]
                Pp = pmm.tile([P, BLK], f32, tag="PC", name=f"P1_{b}")
                for c in range(NCH):
                    nc.tensor.matmul(Pp[0:D, :], i0s[:, c, :], At[:, c, :],
                                     start=(c == 0), stop=(c == NCH - 1))
                    nc.tensor.matmul(Pp[D:2 * D, :], u0s[:, c, :],
                                     Ac[:, c, :],
                                     start=(c == 0), stop=(c == NCH - 1))
                S = psm.tile([P, BLK], f32, tag="S", name=f"S{b}")
                nc.vector.tensor_copy(out=S[:], in_=Pp[:])

                # transpose the [128,128] blocks whole: cols 0:64 become u1
                # natural, cols 64:128 become i1 natural
                PT = ptr.tile([P, NB_CH, P], f32, tag="PT", name=f"PT{b}")
                for f in range(NB_CH):
                    nc.tensor.transpose(PT[:, f, :],
                                        S[:, f * P:(f + 1) * P],
                                        ident[:])
                zi = psm.tile([P, NB_CH, 2 * D], bf16, tag="zi", name=f"zi{b}")
                for f in range(NB_CH):
                    nc.vector.tensor_scalar_mul(out=zi[:, f, 0:D],
                                                in0=PT[:, f, 0:D],
                                                scalar1=rud[:, b, f:f + 1])
                    nc.vector.tensor_scalar_mul(out=zi[:, f, D:2 * D],
                                                in0=PT[:, f, D:2 * D],
                                                scalar1=rid[:, b, f:f + 1])
                # zi is the layer-1 output; zq is its fp8 copy used as the
                # AllGather payload (halves the serialized cc-stream bytes)
                nc.gpsimd.dma_start(out=zi_out[b].ap(), in_=zi[:])
                zq = psm.tile([P, NB_CH, 2 * D], fp8, tag="zq", name=f"zq{b}")
                nc.vector.tensor_copy(out=zq[:], in_=zi[:])
                gi, v = grp_of[b]
                nc.gpsimd.dma_start(out=zin_groups[gi][:, v, :, :],
                                    in_=zq[:])
                state[b] = None

            def gather_group(gi):
                members = groups[gi]
                z_in = zin_groups[gi]
                z_out = pdr.tile([NCORES, P, len(members), NB_CH, 2 * D],
                                 fp8, tag=f"z_out{gi}", name=f"z_out{gi}",
                                 addr_space="Shared")
                nc.gpsimd.collective_compute(
                    "AllGather", ALU.bypass, replica_groups=rg,
                    ins=[z_in.opt()], outs=[z_out.opt()])
                for v, b in enumerate(members):
                    G = pgat.tile([P, NCORES, NB_CH, 2 * D], fp8, tag="G",
                                  name=f"G{b}")
                    nc.sync.dma_start(
                        out=G[:],
                        in_=z_out[:, :, v, :, :].rearrange(
                            "c p f x -> p c f x"))
                    state[b] = G

            def phase2(b):
                At, Ac = at_tiles.pop(b), ac_tiles.pop(b)
                G = state.pop(b)
                Pp = pmm.tile([P, BLK], f32, tag="PC", name=f"P2_{b}")
                for c in range(NCH):
                    nc.tensor.matmul(Pp[0:D, :],
                                     G[:, c // NB_CH, c % NB_CH, D:2 * D],
                                     At[:, c, :],
                                     start=(c == 0), stop=(c == NCH - 1))
                    nc.tensor.matmul(Pp[D:2 * D, :],
                                     G[:, c // NB_CH, c % NB_CH, 0:D],
                                     Ac[:, c, :],
                                     start=(c == 0), stop=(c == NCH - 1))
                T = psm.tile([P, BLK], f32, tag="T", name=f"T{b}")
                nc.vector.tensor_copy(out=T[:], in_=Pp[:])
                nc.scalar.dma_start(out=p2_out[b].ap(), in_=T[:])

            # AG groups: pairs {0,1}/{2,3} balance the per-op cc-stream
            # overhead against how early each group's P2 can start
            if nb >= 3:
                groups = [list(range(nb // 2)), list(range(nb // 2, nb))]
            else:
                groups = [list(range(nb))]
            grp_of = {}
            for gi, members in enumerate(groups):
                for v, b in enumerate(members):
                    grp_of[b] = (gi, v)
            zin_groups = [
                pdr.tile([P, len(members), NB_CH, 2 * D], fp8,
                         tag=f"z_in{gi}", name=f"z_in{gi}")
                for gi, members in enumerate(groups)]
            last_of_group = {members[-1]: gi for gi, members in
                             enumerate(groups)}

            for b in range(nb):
                load(b)
            for b in range(nb):
                phase1(b)
                if b in last_of_group:
                    gather_group(last_of_group[b])
            for b in range(nb):
                phase2(b)
            # warm export last so its sem wait never blocks anything
            nc.scalar.dma_start(out=warm_out.ap(), in_=warm_sb[:])

    nc.compile()
    return nc


# --------------------------------------------------------------------------
# host-side helpers
# --------------------------------------------------------------------------

def _chunk_part(x):
    """[4096, C] -> [128, 32, C] with row = c*128 + p."""
    return np.ascontiguousarray(
        x.reshape(NCH, P, x.shape[1]).transpose(1, 0, 2))


def host_prep_behavior(R):
    """Quantize to e3m4 + exact f64 degree reciprocals."""
    Rq = R.astype(_E3M4)
    deg_u = R.sum(axis=1, dtype=np.float64)
    deg_i = R.sum(axis=0, dtype=np.float64)
    ru = (1.0 / (deg_u + EPS)).astype(np.float32)
    ri = (1.0 / (deg_i + EPS)).astype(np.float32)
    return Rq, ru, ri, deg_u, deg_i


def _core_layouts(Rq, k):
    """Per-core At/Ac tiles in [128, 32, 512] linear-DMA order."""
    rows = Rq[k * BLK:(k + 1) * BLK, :]          # [512, 4096]
    # At[p, ic, u] = rows[u, ic*128+p]
    At = np.ascontiguousarray(
        rows.T.reshape(NCH, P, BLK).transpose(1, 0, 2))
    cols = Rq[:, k * BLK:(k + 1) * BLK]          # [4096, 512]
    # Ac[p, uc, j] = cols[uc*128+p, j]
    Ac = np.ascontiguousarray(
        cols.reshape(NCH, P, BLK).transpose(1, 0, 2))
    return At, Ac


def prep_in_maps(prepped, u0, i0):
    i0s = _chunk_part(i0.astype(_BF16))
    u0s = _chunk_part(u0.astype(_BF16))
    in_maps = []
    for k in range(NCORES):
        sl = slice(k * BLK, (k + 1) * BLK)
        m = {"i0s": i0s, "u0s": u0s}
        # rud[p, b, f] = 1/deg_u[k*512 + f*128 + p] for behavior b
        m["rud"] = np.ascontiguousarray(np.stack(
            [p[1][sl].reshape(NB_CH, P).T for p in prepped], axis=1))
        m["rid"] = np.ascontiguousarray(np.stack(
            [p[2][sl].reshape(NB_CH, P).T for p in prepped], axis=1))
        for b, p in enumerate(prepped):
            At, Ac = _core_layouts(p[0], k)
            m[f"At{b}"] = At
            m[f"Ac{b}"] = Ac
        in_maps.append(m)
    return in_maps


def assemble_dense(results, prepped, nb):
    """Per-behavior (u_acc [N,D], i_acc [N,D]) from per-core outputs."""
    out = []
    for b in range(nb):
        _, ru, ri, _, _ = prepped[b]
        u_acc = np.empty((N, D), np.float32)
        i_acc = np.empty((N, D), np.float32)
        for k in range(NCORES):
            sl = slice(k * BLK, (k + 1) * BLK)
            zi = results[k][f"zi{b}"].astype(np.float32)
            u1 = zi[:, :, 0:D].transpose(1, 0, 2).reshape(BLK, D)
            i1 = zi[:, :, D:2 * D].transpose(1, 0, 2).reshape(BLK, D)
            p2 = results[k][f"p2_{b}"]
            u2 = p2[0:D, :].T * ru[sl][:, None]
            i2 = p2[D:2 * D, :].T * ri[sl][:, None]
            u_acc[sl] = (u1 + u2) * np.float32(0.5)
            i_acc[sl] = (i1 + i2) * np.float32(0.5)
        out.append((u_acc, i_acc))
    return out


def ones_behavior(u0, i0):
    """Analytic LightGCN-2-layer outputs when R is all-ones [N, N]."""
    s_i = i0.astype(np.float64).sum(axis=0)
    s_u = u0.astype(np.float64).sum(axis=0)
    d = N + EPS
    u_row = (s_i / d + s_u * N / (d * d)) * 0.5
    i_row = (s_u / d + s_i * N / (d * d)) * 0.5
    u = np.broadcast_to(u_row.astype(np.float32), (N, D)).copy()
    it = np.broadcast_to(i_row.astype(np.float32), (N, D)).copy()
    return u, it


# --------------------------------------------------------------------------
# cached device runner (compile once per behavior-count, run many)
# --------------------------------------------------------------------------

_RUNNERS = {}


class _Runner:
    def __init__(self, nb):
        self.nb = nb
        self.nc = build_program(nb)
        self._jitted = None
        self._meta = None

    def _prep_jit(self):
        import jax
        import numpy as _np
        from jax.sharding import Mesh, PartitionSpec
        from jax.experimental.shard_map import shard_map
        from concourse import bass2jax
        from concourse.bass2jax import _bass_exec_p, partition_id_tensor
        import concourse.mybir as mybir

        bass2jax.install_neuronx_cc_hook()
        nc = self.nc
        partition_name = (nc.partition_id_tensor.name
                          if nc.partition_id_tensor else None)
        in_names, out_names, out_avals, zero_shapes = [], [], [], []
        for alloc in nc.m.functions[0].allocations:
            if not isinstance(alloc, mybir.MemoryLocationSet):
                continue
            name = alloc.memorylocations[0].name
            if alloc.kind == "ExternalInput":
                if name != partition_name:
                    in_names.append(name)
            elif alloc.kind == "ExternalOutput":
                shape = tuple(alloc.tensor_shape)
                dtype = mybir.dt.np(alloc.dtype)
                out_names.append(name)
                out_avals.append(jax.core.ShapedArray(shape, dtype))
                zero_shapes.append((shape, dtype))
        n_params = len(in_names)
        full_in_names = list(in_names) + list(out_names)
        if partition_name is not None:
            full_in_names.append(partition_name)

        def _body(*args):
            operands = list(args)
            if partition_name is not None:
                operands.append(partition_id_tensor())
            outs = _bass_exec_p.bind(
                *operands,
                out_avals=tuple(out_avals),
                in_names=tuple(full_in_names),
                out_names=tuple(out_names),
                lowering_input_output_aliases=(),
                sim_require_finite=True,
                sim_require_nnan=True,
                nc=nc,
            )
            return tuple(outs)

        devices = jax.devices()[:NCORES]
        mesh = Mesh(_np.asarray(devices), ("core",))
        n_outs = len(out_names)
        in_specs = (PartitionSpec("core"),) * (n_params + n_outs)
        out_specs = (PartitionSpec("core"),) * n_outs
        donate = tuple(range(n_params, n_params + n_outs))
        self._jitted = jax.jit(
            shard_map(_body, mesh=mesh, in_specs=in_specs,
                      out_specs=out_specs, check_rep=False),
            donate_argnums=donate, keep_unused=True)
        self._meta = (in_names, out_names, out_avals, zero_shapes, n_params)

    def run(self, in_maps):
        if self._jitted is None:
            self._prep_jit()
        import numpy as _np
        in_names, out_names, out_avals, zero_shapes, n_params = self._meta
        concat_in = [
            _np.concatenate([_np.asarray(in_maps[c][nm]) for c in range(NCORES)],
                            axis=0)
            for nm in in_names]
        concat_zeros = [_np.zeros((NCORES * s[0], *s[1:]), dt)
                        for (s, dt) in zero_shapes]
        out_arrs = self._jitted(*concat_in, *concat_zeros)
        results = []
        for c in range(NCORES):
            results.append({
                nm: _np.asarray(out_arrs[i]).reshape(
                    NCORES, *out_avals[i].shape)[c]
                for i, nm in enumerate(out_names)})
        return results

    def run_traced(self, in_maps, tmpdir=None):
        """Run through run_bass_kernel_spmd with NTFF tracing (recompiles)."""
        _install_trace_shims()
        from concourse.bass_utils import run_bass_kernel_spmd
        return run_bass_kernel_spmd(self.nc, in_maps,
                                    core_ids=list(range(NCORES)),
                                    trace=True, tmpdir=tmpdir)


def _install_trace_shims():
    """This image's antenv lacks axon_hooks (the NTFF-hook registry) and has
    no artifact bucket; recreate the hook from the boot recipe and make
    artifact upload a local no-op."""
    import sys, types, importlib.util

    if "antenv.axon_hooks" not in sys.modules:
        mod = types.ModuleType("antenv.axon_hooks")
        mod._hook = None

        def set_axon_ntff_profile_hook(h):
            mod._hook = h

        def get_axon_ntff_profile_hook():
            return mod._hook

        mod.set_axon_ntff_profile_hook = set_axon_ntff_profile_hook
        mod.get_axon_ntff_profile_hook = get_axon_ntff_profile_hook
        import antenv
        sys.modules["antenv.axon_hooks"] = mod
        antenv.axon_hooks = mod

        spec = importlib.util.spec_from_file_location(
            "trn_boot_shim", "/root/.axon_site/trn_agent_boot/trn_boot.py")
        boot = importlib.util.module_from_spec(spec)
        spec.loader.exec_module(boot)
        hook = boot._ntff_profile_via_ctypes("/opt/axon/libaxon_pjrt.so")
        mod._hook = hook

    import concourse.bass_utils as bu
    if not getattr(bu.upload_artifacts, "_is_local_shim", False):
        def _local_upload(tmpdir):
            return tmpdir
        _local_upload._is_local_shim = True
        bu.upload_artifacts = _local_upload


def get_runner(nb):
    if nb not in _RUNNERS:
        _RUNNERS[nb] = _Runner(nb)
    return _RUNNERS[nb]


# --------------------------------------------------------------------------
# entry point
# --------------------------------------------------------------------------

def _is_ones(a):
    return a[0, 0] == 1.0 and bool(np.all(a == np.float32(1.0)))


def kernel(**inputs):
    inputs = {k: np.asarray(v) for k, v in inputs.items()}
    u0 = np.ascontiguousarray(inputs["user_embedding"], dtype=np.float32)
    i0 = np.ascontiguousarray(inputs["item_embedding"], dtype=np.float32)

    real_names = ["R_click", "R_fav", "R_cart", "R_buy"]
    virt_names = [("M_click", "add_click"), ("M_fav", "add_fav"),
                  ("M_cart", "add_cart")]
    mats = [np.asarray(inputs[n], dtype=np.float32) for n in real_names]
    mats += [np.asarray(inputs[m], dtype=np.float32) for m, _ in virt_names]

    dense_idx = [j for j, a in enumerate(mats) if not _is_ones(a)]
    per_behavior = [None] * 7

    if dense_idx:
        nb = len(dense_idx)
        runner = get_runner(nb)
        prepped = [host_prep_behavior(mats[j]) for j in dense_idx]
        in_maps = prep_in_maps(prepped, u0, i0)
        results = runner.run(in_maps)
        dense = assemble_dense(results, prepped, nb)
        for pos, j in enumerate(dense_idx):
            per_behavior[j] = dense[pos]

    ones_cache = None
    for j, a in enumerate(mats):
        if per_behavior[j] is None:
            if ones_cache is None:
                ones_cache = ones_behavior(u0, i0)
            per_behavior[j] = ones_cache
    ur = [per_behavior[j][0] for j in range(4)]
    ir = [per_behavior[j][1] for j in range(4)]
    uv = [per_behavior[4 + j][0] + np.asarray(inputs[virt_names[j][1]],
                                              dtype=np.float32)
          for j in range(3)]
    iv = [per_behavior[4 + j][1] for j in range(3)]

    out = np.concatenate(
        [np.stack(ur), np.stack(ir), np.stack(uv), np.stack(iv)], axis=0)
    return np.ascontiguousarray(out, dtype=np.float32)
